# revision 1
# baseline (speedup 1.0000x reference)
"""Trainium2 Bass kernel for nn_CapacitanceMatrix.

C[b, i, j] = sigmoid(x[b]·Wd[i] + bd[i])        if i == j
           = -softplus(x[b]·Wo[m] + bo[m])      if i != j  (m = row-major off-diag idx)

Strategy: fold the scatter into the weight layout. Build W_full (256, D) whose
row p = i*16+j is Wd[i] (diag) or Wo[m] (off-diag), so the matmul output row is
already the flattened (16, 16) matrix. Pure data parallel over 8 cores: each
core gets 8192 rows of x, pre-transposed on host to xT (D, 8192) so the
contraction dim sits on SBUF partitions with contiguous DMA.

Per core: for each 128-row batch tile, accumulate 8 matmuls over D-chunks
(lhsT = xT chunk stationary, rhs = W_full^T (128, 256) moving) into PSUM after
seeding it with the bias via a K=1 ones x bias matmul. Epilogue: softplus
(ScalarE LUT) -> negate (VectorE) -> sigmoid overwrite of the 16 diagonal
columns (stride-17 AP). Output rows DMA out contiguously.
"""

import os
import sys

sys.path.insert(0, "/opt/trn_rl_repo")

from contextlib import ExitStack

import numpy as np

import concourse.bass as bass
import concourse.tile as tile
from concourse import bacc, mybir
from concourse.bass_utils import run_bass_kernel_spmd

B = 65536
D = 1024
K = 16
NOUT = K * K  # 256
NCORES = 8
BC = B // NCORES  # 8192 rows per core
KD = D // 128  # 8 contraction chunks
BLK = 1024  # batch columns loaded per block
OJ = 2  # j-subtiles batched per output DMA
CONST_F = KD * NOUT + 2 * NOUT + 128  # wt chunks + doubled bias + ones

# matmul dtype: "bfloat16" (fp32 PSUM accumulate; ~4e-3 scale-relative absmax,
# ~105us) or "float32r" (~2.3e-4, ~170us) or "float32" (~1.6e-5, ~285us)
MM_DT_NAME = os.environ.get("CAP_MM_DT", "bfloat16")

_CACHE = {}

_ACT_TABLES_PATCHED = False


def _pin_act_table_set():
    """Force Exp and Ln to resolve to the single LUT set that holds both
    (`natural_log_exp_and_others`), so the epilogue's exp->ln alternation
    doesn't thrash ACT_TABLE_LOADs (measured: 54 loads, 69us of ScalarE)."""
    global _ACT_TABLES_PATCHED
    if _ACT_TABLES_PATCHED:
        return
    import concourse.hw_specs as hw_specs

    orig = hw_specs.get_activation_tables

    def patched(arch):
        tables = {k: set(v) for k, v in orig(arch).items()}
        keep = "natural_log_exp_and_others"
        if keep in tables:
            for k, v in tables.items():
                if k != keep:
                    v.discard(mybir.ActivationFunctionType.Exp)
                    v.discard(mybir.ActivationFunctionType.Ln)
        return tables

    bacc.get_activation_tables = patched
    _ACT_TABLES_PATCHED = True


def _mm_dt():
    return getattr(mybir.dt, MM_DT_NAME)


def _np_dt():
    return mybir.dt.np(_mm_dt())


def _build_bass():
    _pin_act_table_set()
    mm_dt = _mm_dt()
    f32 = mybir.dt.float32
    nc = bacc.Bacc("TRN2", target_bir_lowering=False, debug=False)
    # x pre-tiled on host: [block, chunk, partition, col] so every chunk-block
    # DMA is one fully contiguous DRAM read
    xT = nc.dram_tensor(
        "xT", [BC // BLK, KD, 128, BLK], mm_dt, kind="ExternalInput"
    ).ap()
    # const blob: cols 0:2048 = wt chunks, row 0 extras: doubled bias + ones
    consts = nc.dram_tensor("consts", [128, CONST_F], mm_dt, kind="ExternalInput").ap()
    out = nc.dram_tensor("out", [BC, NOUT], f32, kind="ExternalOutput").ap()

    with tile.TileContext(nc) as tc, ExitStack() as ctx:
        const_pool = ctx.enter_context(tc.tile_pool(name="const", bufs=1))
        x_pool = ctx.enter_context(tc.tile_pool(name="x", bufs=4 * KD))
        out_pool = ctx.enter_context(tc.tile_pool(name="o", bufs=6))
        psum_pool = ctx.enter_context(tc.tile_pool(name="ps", bufs=6, space="PSUM"))

        const_sb = const_pool.tile([128, CONST_F], mm_dt)
        # bias/ones first (the seed matmul is the first consumer), then wt
        # chunks in parallel (Bacc legalizes multi-wait consumers)
        nc.scalar.dma_start(
            const_sb[0:1, KD * NOUT :], consts[0:1, KD * NOUT :]
        )
        for c in range(KD):
            nc.scalar.dma_start(
                const_sb[:, c * NOUT : (c + 1) * NOUT],
                consts[:, c * NOUT : (c + 1) * NOUT],
            )
        wt_sb = [const_sb[:, c * NOUT : (c + 1) * NOUT] for c in range(KD)]
        bias2_sb = const_sb[0:1, KD * NOUT : KD * NOUT + 2 * NOUT]
        ones_sb = const_sb[0:1, KD * NOUT + 2 * NOUT : KD * NOUT + 2 * NOUT + 128]

        for blk in range(BC // BLK):
            # one tile per D-chunk so each matmul waits on exactly one DMA
            x_sb = []
            for c in range(KD):
                xc = x_pool.tile([128, BLK], mm_dt, tag="x")
                nc.sync.dma_start(xc[:], xT[blk, c])
                x_sb.append(xc)
            for jg in range(BLK // (128 * OJ)):
                # one out tile covers OJ j-subtiles -> one big out-DMA
                ot = out_pool.tile([128, OJ, NOUT], f32, tag="ot")
                for pj in range(OJ // 2):
                    # a pair of j-subtiles shares one full PSUM bank so the
                    # epilogue runs 512-wide (halves per-op access latency)
                    oj0 = pj * 2
                    ps = psum_pool.tile([128, 2, NOUT], f32)
                    # seed both halves with the (doubled) bias row
                    nc.tensor.matmul(
                        ps[:],
                        lhsT=ones_sb,
                        rhs=bias2_sb.rearrange("a (q n) -> a q n", q=2),
                        start=True,
                        stop=False,
                    )
                    for jj in range(2):
                        j = jg * OJ + oj0 + jj
                        for c in range(KD):
                            nc.tensor.matmul(
                                ps[:, jj, :],
                                lhsT=x_sb[c][:, bass.ts(j, 128)],
                                rhs=wt_sb[c],
                                start=False,
                                stop=(jj == 1 and c == KD - 1),
                                skip_group_check=True,
                            )
                    # Scalar LUT set has exp+ln but no softplus/sigmoid:
                    #   off-diag: -softplus(z) = -ln(1 + e^z)
                    #   diag: host negated Wd rows, so psum holds -z and
                    #         sigmoid(z) = 1/(1 + e^-z) = 1/(1 + E_diag)
                    ev = out_pool.tile([128, 2, NOUT], f32, tag="ev")
                    nc.scalar.activation(
                        ev[:], ps[:], mybir.ActivationFunctionType.Exp
                    )
                    nc.scalar.activation(
                        ot[:, oj0 : oj0 + 2, :],
                        ev[:],
                        mybir.ActivationFunctionType.Ln,
                        bias=1.0,
                    )
                    nc.vector.tensor_scalar_mul(
                        ot[:, oj0 : oj0 + 2, :], ot[:, oj0 : oj0 + 2, :], -1.0
                    )
                    dtmp = out_pool.tile([128, 2, K], f32, tag="dtmp")
                    nc.vector.tensor_scalar_add(dtmp[:], ev[:, :, ::17], 1.0)
                    nc.vector.reciprocal(ot[:, oj0 : oj0 + 2, ::17], dtmp[:])
                # dest rows r0+oj*128+p for tile element (p, oj, n)
                r0 = blk * BLK + jg * 128 * OJ
                dst = out[r0 : r0 + 128 * OJ, :].rearrange(
                    "(oj p) n -> p oj n", p=128
                )
                nc.sync.dma_start(dst, ot[:])
    nc.compile()
    return nc


def _get_nc():
    key = MM_DT_NAME
    if key not in _CACHE:
        _CACHE[key] = _build_bass()
    return _CACHE[key]


def _host_prep(x, Wd, bd, Wo, bo):
    np_dt = _np_dt()
    off_i, off_j = np.nonzero(~np.eye(K, dtype=bool))
    w_full = np.empty((NOUT, D), np.float32)
    b_full = np.empty(NOUT, np.float32)
    w_full[off_i * K + off_j] = Wo
    b_full[off_i * K + off_j] = bo
    # diag rows negated: device computes sigmoid(z) as 1/(1 + exp(-z))
    diag_pos = np.arange(K) * (K + 1)
    w_full[diag_pos] = -Wd
    b_full[diag_pos] = -bd
    wt = w_full.T  # (D, 256)
    # const blob layout must match const_sb: [128, CONST_F]
    consts = np.zeros((128, CONST_F), np.float32)
    # wt_sb chunk c at cols [c*256, (c+1)*256): consts[p, c*256+n] = wt[c*128+p, n]
    consts[:, : KD * NOUT] = wt.reshape(KD, 128, NOUT).transpose(1, 0, 2).reshape(
        128, KD * NOUT
    )
    consts[0, KD * NOUT : KD * NOUT + NOUT] = b_full
    consts[0, KD * NOUT + NOUT : KD * NOUT + 2 * NOUT] = b_full
    consts[0, KD * NOUT + 2 * NOUT : KD * NOUT + 2 * NOUT + 128] = 1.0
    consts = np.ascontiguousarray(consts).astype(np_dt)
    nblk = BC // BLK
    in_maps = []
    for c in range(NCORES):
        xs = x[c * BC : (c + 1) * BC]  # (BC, D)
        # -> (nblk, KD, 128, BLK): element (b, kd, p, t) = xs[b*BLK+t, kd*128+p]
        xT = np.ascontiguousarray(
            xs.reshape(nblk, BLK, KD, 128).transpose(0, 2, 3, 1)
        ).astype(np_dt)
        in_maps.append({"xT": xT, "consts": consts})
    return in_maps


def _install_env_shims():
    """The agent image's `antenv` stub lacks `axon_hooks`; bass_utils imports
    it on any trace=True/BASS_TRACE run. Provide it (wired to the ctypes NTFF
    hook when available), and skip the S3 artifact upload (no egress)."""
    if "antenv.axon_hooks" in sys.modules:
        return
    import types

    try:
        import antenv
    except ImportError:
        return
    if hasattr(antenv, "axon_hooks"):
        return
    mod = types.ModuleType("antenv.axon_hooks")
    hook = [None]
    try:
        from trn_agent_boot.trn_boot import _ntff_profile_via_ctypes

        hook[0] = _ntff_profile_via_ctypes("/opt/axon/libaxon_pjrt.so")
    except Exception:
        pass
    mod.set_axon_ntff_profile_hook = lambda h: hook.__setitem__(0, h)
    mod.get_axon_ntff_profile_hook = lambda: hook[0]
    sys.modules["antenv.axon_hooks"] = mod
    antenv.axon_hooks = mod

    import concourse.bass_utils as bu

    bu.upload_artifacts = lambda tmpdir: tmpdir


def _run(in_maps, **kwargs):
    _install_env_shims()
    nc = _get_nc()
    return run_bass_kernel_spmd(nc, in_maps, list(range(NCORES)), **kwargs)


def kernel(x, Wd, bd, Wo, bo, _bench_results=None, **kwargs):
    x = np.asarray(x, np.float32)
    in_maps = _host_prep(
        x,
        np.asarray(Wd, np.float32),
        np.asarray(bd, np.float32),
        np.asarray(Wo, np.float32),
        np.asarray(bo, np.float32),
    )
    res = _run(in_maps, **kwargs)
    if _bench_results is not None:
        _bench_results.append(res)
    outs = [res.results[c]["out"] for c in range(NCORES)]
    return np.concatenate(outs, axis=0).reshape(B, K, K)



# revision 5
# speedup vs baseline: 1.1089x; 1.1089x over previous
"""Trainium2 Bass kernel for nn_CapacitanceMatrix — v2 (transposed matmul).

C[b, i, j] = sigmoid(x[b]·Wd[i] + bd[i])        if i == j
           = -softplus(x[b]·Wo[m] + bo[m])      if i != j  (m = row-major off-diag idx)

v2 design (v1 baseline: 110.5us, x-stationary matmuls + f32 output):
- Transposed matmul: the 256 fused weight rows ([240 off-diag | 16 negated
  diag]) are the stationary lhsT in 128-row halves; x streams through as the
  512-wide moving operand. Output lands as [outs, batch] on PSUM partitions.
  No bias-seed matmuls: the bias rides the Exp activation's per-partition
  bias AP. 256 matmuls/core at 512-col streams vs 544 at 256.
- Epilogue per 1024-batch pair and 128-row half (psum = 2 banks):
    ev = exp(psum + b)       ScalarE, bf16 out
    off:  ot = -ln(1 + ev)   ScalarE Ln(bias=1) then DVE negate (bf16 2x)
    diag: ot = 1 / (1 + ev)  DVE add1 + reciprocal (psum holds -z)
- Output stays transposed in DRAM as [256, 8192] bf16 per core (2KB DMA
  rows); host gathers rows back to (i, j) order, casts f32, transposes.
- DMA: x fully SBUF-resident (16MB/core), 4KB input packets, separate rings
  for input (sync) / weights+bias (scalar) / output (gpsimd) so the output
  stream never head-of-line blocks input prefetch.
HBM traffic 21.3MB/core (16.8 in + 4.2 out) vs 25.8MB in v1.
"""

import sys

sys.path.insert(0, "/opt/trn_rl_repo")

from contextlib import ExitStack

import numpy as np

import concourse.bass as bass  # noqa: F401  (kept for parity with framework imports)
import concourse.tile as tile
from concourse import bacc, mybir
from concourse.bass_utils import run_bass_kernel_spmd

B = 65536
D = 1024
K = 16
NOUT = K * K  # 256
NCORES = 8
BC = B // NCORES  # 8192 rows per core
KD = D // 128  # 8 contraction chunks
BLKC = 2048  # batch columns per x block (4KB bf16 DMA rows)
NBLK = BC // BLKC  # 4
NPAIR = BC // 1024  # 8 epilogue pairs per core
NDIAG_P0 = 240 - 128  # partition where diag rows start in half B (112)

MM_DT_NAME = "bfloat16"  # kept for test.py compat

_CACHE = {}

_ACT_TABLES_PATCHED = False


def _pin_act_table_set():
    """Force Exp and Ln to resolve to the single LUT set that holds both
    (`natural_log_exp_and_others`) so the exp->ln alternation never thrashes
    ACT_TABLE_LOADs."""
    global _ACT_TABLES_PATCHED
    if _ACT_TABLES_PATCHED:
        return
    import concourse.hw_specs as hw_specs

    orig = hw_specs.get_activation_tables

    def patched(arch):
        tables = {k: set(v) for k, v in orig(arch).items()}
        keep = "natural_log_exp_and_others"
        if keep in tables:
            for k, v in tables.items():
                if k != keep:
                    v.discard(mybir.ActivationFunctionType.Exp)
                    v.discard(mybir.ActivationFunctionType.Ln)
        return tables

    bacc.get_activation_tables = patched
    _ACT_TABLES_PATCHED = True


def _build_bass():
    _pin_act_table_set()
    bf16 = mybir.dt.bfloat16
    f32 = mybir.dt.float32
    nc = bacc.Bacc("TRN2", target_bir_lowering=False, debug=False)
    # x pre-tiled on host: [block, chunk, partition, col]; every chunk DMA is
    # 128 fully contiguous 4KB partition rows
    xT = nc.dram_tensor("xT", [NBLK, KD, 128, BLKC], bf16, kind="ExternalInput").ap()
    # wts[p, c*256 + n] = W_dev[n, c*128 + p]; W_dev = [Wo; -Wd]
    wts = nc.dram_tensor("wts", [128, KD * NOUT], bf16, kind="ExternalInput").ap()
    # biasv[p, h] = bias for output row h*128+p ([bo; -bd])
    biasv = nc.dram_tensor("biasv", [128, 2], f32, kind="ExternalInput").ap()
    # transposed output: row r = fused weight row, col = batch index in core
    outT = nc.dram_tensor("outT", [NOUT, BC], bf16, kind="ExternalOutput").ap()

    with tile.TileContext(nc) as tc, ExitStack() as ctx:
        const_pool = ctx.enter_context(tc.tile_pool(name="const", bufs=1))
        x_pool = ctx.enter_context(tc.tile_pool(name="x", bufs=NBLK * KD))
        ev_pool = ctx.enter_context(tc.tile_pool(name="ev", bufs=4))
        sp_pool = ctx.enter_context(tc.tile_pool(name="sp", bufs=4))
        ot_pool = ctx.enter_context(tc.tile_pool(name="ot", bufs=4))
        dt_pool = ctx.enter_context(tc.tile_pool(name="dt", bufs=4))
        psum_pool = ctx.enter_context(tc.tile_pool(name="ps", bufs=4, space="PSUM"))

        wt_sb = const_pool.tile([128, KD * NOUT], bf16)
        bias_sb = const_pool.tile([128, 2], f32)
        nc.scalar.dma_start(wt_sb[:], wts)
        nc.scalar.dma_start(bias_sb[:], biasv)

        for blk in range(NBLK):
            xs = []
            for c in range(KD):
                xc = x_pool.tile([128, BLKC], bf16, tag="x")
                nc.sync.dma_start(xc[:], xT[blk, c])
                xs.append(xc)
            for hp in range(BLKC // 1024):
                pp = blk * (BLKC // 1024) + hp
                col0 = hp * 1024
                for half in range(2):
                    ps = psum_pool.tile([128, 1024], f32)
                    for c in range(KD):
                        lhsT = wt_sb[
                            :, c * NOUT + half * 128 : c * NOUT + half * 128 + 128
                        ]
                        for g in range(2):
                            nc.tensor.matmul(
                                ps[:, g * 512 : (g + 1) * 512],
                                lhsT=lhsT,
                                rhs=xs[c][:, col0 + g * 512 : col0 + (g + 1) * 512],
                                start=(c == 0),
                                stop=(c == KD - 1),
                                skip_group_check=True,
                            )
                    ev = ev_pool.tile([128, 1024], bf16, tag="ev")
                    nc.scalar.activation(
                        ev[:],
                        ps[:],
                        mybir.ActivationFunctionType.Exp,
                        bias=bias_sb[:, half : half + 1],
                    )
                    ot = ot_pool.tile([128, 1024], bf16, tag="ot")
                    sp = sp_pool.tile([128, 1024], bf16, tag="sp")
                    cols = slice(pp * 1024, (pp + 1) * 1024)
                    if half == 0:
                        nc.scalar.activation(
                            sp[:], ev[:], mybir.ActivationFunctionType.Ln, bias=1.0
                        )
                        nc.vector.tensor_scalar_mul(ot[:], sp[:], -1.0)
                        nc.gpsimd.dma_start(outT[0:128, cols], ot[:])
                    else:
                        # compute-engine APs need 32-aligned partition starts,
                        # so the sigmoid runs on [96:128] (96..111 is junk that
                        # is never DMA'd out); DMA APs have no such limit
                        p0 = NDIAG_P0  # 112
                        nc.scalar.activation(
                            sp[0:p0],
                            ev[0:p0],
                            mybir.ActivationFunctionType.Ln,
                            bias=1.0,
                        )
                        nc.vector.tensor_scalar_mul(ot[0:p0], sp[0:p0], -1.0)
                        dt_ = dt_pool.tile([128, 1024], bf16, tag="dt")
                        sc = dt_pool.tile([128, 1024], bf16, tag="sc")
                        nc.vector.tensor_scalar_add(dt_[96:128], ev[96:128], 1.0)
                        with nc.allow_low_precision(
                            reason="sigmoid in bf16: 2^-9 rel err vs 2e-2 budget"
                        ):
                            nc.vector.reciprocal(sc[96:128], dt_[96:128])
                        nc.gpsimd.dma_start(outT[128 : 128 + p0, cols], ot[0:p0])
                        nc.gpsimd.dma_start(
                            outT[128 + p0 : 256, cols], sc[p0:128]
                        )
    nc.compile()
    return nc


def _get_nc():
    if "nc" not in _CACHE:
        _CACHE["nc"] = _build_bass()
    return _CACHE["nc"]


def _host_prep(x, Wd, bd, Wo, bo):
    import ml_dtypes

    np_bf16 = ml_dtypes.bfloat16
    # fused rows: [Wo (240) ; -Wd (16)] — diag negated so psum holds -z and
    # sigmoid(z) = 1/(1 + e^-z) comes out of the shared exp pass
    w_dev = np.concatenate([Wo, -Wd], axis=0)  # (256, D)
    b_dev = np.concatenate([bo, -bd], axis=0)  # (256,)
    wts = np.ascontiguousarray(
        w_dev.T.reshape(KD, 128, NOUT).transpose(1, 0, 2).reshape(128, KD * NOUT)
    ).astype(np_bf16)
    biasv = np.ascontiguousarray(
        np.stack([b_dev[0:128], b_dev[128:256]], axis=1)
    ).astype(np.float32)
    in_maps = []
    for c in range(NCORES):
        xs = x[c * BC : (c + 1) * BC]  # (BC, D)
        # -> (NBLK, KD, 128, BLKC): elem (blk, kd, p, t) = xs[blk*BLKC+t, kd*128+p]
        xTc = np.ascontiguousarray(
            xs.reshape(NBLK, BLKC, KD, 128).transpose(0, 2, 3, 1)
        ).astype(np_bf16)
        in_maps.append({"xT": xTc, "wts": wts, "biasv": biasv})
    return in_maps


def _install_env_shims():
    """The agent image's `antenv` stub lacks `axon_hooks`; bass_utils imports
    it on any trace=True/BASS_TRACE run. Provide it (wired to the ctypes NTFF
    hook when available), and skip the S3 artifact upload (no egress)."""
    if "antenv.axon_hooks" in sys.modules:
        return
    import types

    try:
        import antenv
    except ImportError:
        return
    if hasattr(antenv, "axon_hooks"):
        return
    mod = types.ModuleType("antenv.axon_hooks")
    hook = [None]
    try:
        from trn_agent_boot.trn_boot import _ntff_profile_via_ctypes

        hook[0] = _ntff_profile_via_ctypes("/opt/axon/libaxon_pjrt.so")
    except Exception:
        pass
    mod.set_axon_ntff_profile_hook = lambda h: hook.__setitem__(0, h)
    mod.get_axon_ntff_profile_hook = lambda: hook[0]
    sys.modules["antenv.axon_hooks"] = mod
    antenv.axon_hooks = mod

    import concourse.bass_utils as bu

    bu.upload_artifacts = lambda tmpdir: tmpdir


def _run(in_maps, **kwargs):
    _install_env_shims()
    nc = _get_nc()
    return run_bass_kernel_spmd(nc, in_maps, list(range(NCORES)), **kwargs)


# row r of outT -> flat (i, j) position: P[i*16+j] = source row
def _out_perm():
    off_i, off_j = np.nonzero(~np.eye(K, dtype=bool))
    P = np.empty(NOUT, np.int64)
    P[off_i * K + off_j] = np.arange(K * (K - 1))
    P[np.arange(K) * (K + 1)] = K * (K - 1) + np.arange(K)
    return P


def kernel(x, Wd, bd, Wo, bo, _bench_results=None, **kwargs):
    x = np.asarray(x, np.float32)
    in_maps = _host_prep(
        x,
        np.asarray(Wd, np.float32),
        np.asarray(bd, np.float32),
        np.asarray(Wo, np.float32),
        np.asarray(bo, np.float32),
    )
    res = _run(in_maps, **kwargs)
    if _bench_results is not None:
        _bench_results.append(res)
    P = _out_perm()
    out = np.empty((B, NOUT), np.float32)
    for c in range(NCORES):
        oT = np.asarray(res.results[c]["outT"], dtype=np.float32)  # (256, BC)
        out[c * BC : (c + 1) * BC] = oT[P].T
    return out.reshape(B, K, K)


# revision 7
# speedup vs baseline: 1.2859x; 1.1597x over previous
"""Trainium2 Bass kernel for nn_CapacitanceMatrix — v2 (transposed matmul).

C[b, i, j] = sigmoid(x[b]·Wd[i] + bd[i])        if i == j
           = -softplus(x[b]·Wo[m] + bo[m])      if i != j  (m = row-major off-diag idx)

v2 design (v1 baseline: 110.5us, x-stationary matmuls + f32 output):
- Transposed matmul: the 256 fused weight rows ([240 off-diag | 16 negated
  diag]) are the stationary lhsT in 128-row halves; x streams through as the
  512-wide moving operand. Output lands as [outs, batch] on PSUM partitions.
  No bias-seed matmuls: the bias rides the Exp activation's per-partition
  bias AP. 256 matmuls/core at 512-col streams vs 544 at 256.
- Epilogue per 1024-batch pair and 128-row half (psum = 2 banks):
    ev = exp(psum + b)       ScalarE, bf16 out
    off:  ot = -ln(1 + ev)   ScalarE Ln(bias=1) then DVE negate (bf16 2x)
    diag: ot = 1 / (1 + ev)  DVE add1 + reciprocal (psum holds -z)
- Output stays transposed in DRAM as [256, 8192] bf16 per core (2KB DMA
  rows); host gathers rows back to (i, j) order, casts f32, transposes.
- DMA: x fully SBUF-resident (16MB/core), 4KB input packets, separate rings
  for input (sync) / weights+bias (scalar) / output (gpsimd) so the output
  stream never head-of-line blocks input prefetch.
HBM traffic 21.3MB/core (16.8 in + 4.2 out) vs 25.8MB in v1.
"""

import sys

sys.path.insert(0, "/opt/trn_rl_repo")

from contextlib import ExitStack

import numpy as np

import concourse.bass as bass  # noqa: F401  (kept for parity with framework imports)
import concourse.tile as tile
from concourse import bacc, mybir
from concourse.bass_utils import run_bass_kernel_spmd

B = 65536
D = 1024
K = 16
NOUT = K * K  # 256
NCORES = 8
BC = B // NCORES  # 8192 rows per core
KD = D // 128  # 8 contraction chunks
BLKC = 2048  # batch columns per x block (4KB bf16 DMA rows)
NBLK = BC // BLKC  # 4
NPAIR = BC // 1024  # 8 epilogue pairs per core
NDIAG_P0 = 240 - 128  # partition where diag rows start in half B (112)

MM_DT_NAME = "bfloat16"  # kept for test.py compat

_CACHE = {}

_ACT_TABLES_PATCHED = False


def _pin_act_table_set():
    """Force Exp and Ln to resolve to the single LUT set that holds both
    (`natural_log_exp_and_others`) so the exp->ln alternation never thrashes
    ACT_TABLE_LOADs."""
    global _ACT_TABLES_PATCHED
    if _ACT_TABLES_PATCHED:
        return
    import concourse.hw_specs as hw_specs

    orig = hw_specs.get_activation_tables

    def patched(arch):
        tables = {k: set(v) for k, v in orig(arch).items()}
        keep = "natural_log_exp_and_others"
        if keep in tables:
            for k, v in tables.items():
                if k != keep:
                    v.discard(mybir.ActivationFunctionType.Exp)
                    v.discard(mybir.ActivationFunctionType.Ln)
        return tables

    bacc.get_activation_tables = patched
    _ACT_TABLES_PATCHED = True


def _build_bass():
    _pin_act_table_set()
    bf16 = mybir.dt.bfloat16
    f32 = mybir.dt.float32
    nc = bacc.Bacc("TRN2", target_bir_lowering=False, debug=False)
    # x pre-tiled on host: [block, chunk, partition, col]; every chunk DMA is
    # 128 fully contiguous 4KB partition rows
    xT = nc.dram_tensor("xT", [NBLK, KD, 128, BLKC], bf16, kind="ExternalInput").ap()
    # wts[p, c*256 + n] = W_dev[n, c*128 + p]; W_dev = [Wo; -Wd]
    wts = nc.dram_tensor("wts", [128, KD * NOUT], bf16, kind="ExternalInput").ap()
    # biasv[p, h] = bias for output row h*128+p ([bo; -bd])
    biasv = nc.dram_tensor("biasv", [128, 2], f32, kind="ExternalInput").ap()
    # transposed output: row r = fused weight row, col = batch index in core
    outT = nc.dram_tensor("outT", [NOUT, BC], bf16, kind="ExternalOutput").ap()

    with tile.TileContext(nc) as tc, ExitStack() as ctx:
        const_pool = ctx.enter_context(tc.tile_pool(name="const", bufs=1))
        x_pool = ctx.enter_context(tc.tile_pool(name="x", bufs=NBLK * KD))
        ev_pool = ctx.enter_context(tc.tile_pool(name="ev", bufs=4))
        sp_pool = ctx.enter_context(tc.tile_pool(name="sp", bufs=4))
        ot_pool = ctx.enter_context(tc.tile_pool(name="ot", bufs=4))
        dt_pool = ctx.enter_context(tc.tile_pool(name="dt", bufs=2))
        psum_pool = ctx.enter_context(tc.tile_pool(name="ps", bufs=4, space="PSUM"))

        wt_sb = const_pool.tile([128, KD * NOUT], bf16)
        bias_sb = const_pool.tile([128, 2], f32)
        nc.scalar.dma_start(wt_sb[:], wts)
        nc.scalar.dma_start(bias_sb[:], biasv)

        for blk in range(NBLK):
            xs = []
            for c in range(KD):
                xc = x_pool.tile([128, BLKC], bf16, tag="x")
                nc.sync.dma_start(xc[:], xT[blk, c])
                xs.append(xc)
            for hp in range(BLKC // 1024):
                pp = blk * (BLKC // 1024) + hp
                col0 = hp * 1024
                for half in range(2):
                    ps = psum_pool.tile([128, 1024], f32)
                    for c in range(KD):
                        lhsT = wt_sb[
                            :, c * NOUT + half * 128 : c * NOUT + half * 128 + 128
                        ]
                        for g in range(2):
                            nc.tensor.matmul(
                                ps[:, g * 512 : (g + 1) * 512],
                                lhsT=lhsT,
                                rhs=xs[c][:, col0 + g * 512 : col0 + (g + 1) * 512],
                                start=(c == 0),
                                stop=(c == KD - 1),
                                skip_group_check=True,
                            )
                    ev = ev_pool.tile([128, 1024], bf16, tag="ev")
                    nc.scalar.activation(
                        ev[:],
                        ps[:],
                        mybir.ActivationFunctionType.Exp,
                        bias=bias_sb[:, half : half + 1],
                    )
                    ot = ot_pool.tile([128, 1024], bf16, tag="ot")
                    sp = sp_pool.tile([128, 1024], bf16, tag="sp")
                    cols = slice(pp * 1024, (pp + 1) * 1024)
                    if half == 0:
                        nc.scalar.activation(
                            sp[:], ev[:], mybir.ActivationFunctionType.Ln, bias=1.0
                        )
                        nc.vector.tensor_scalar_mul(ot[:], sp[:], -1.0)
                        nc.gpsimd.dma_start(outT[0:128, cols], ot[:])
                    else:
                        # diag rows sit at [112:128] where psum held -z, so
                        # sp = ln(1+e^-z) = softplus(-z) there and
                        # sigmoid(z) = exp(-sp). DVE reciprocal measured
                        # 6.5us/op on hw — this keeps the whole path on
                        # ScalarE. Compute-engine APs need 32-aligned
                        # partition starts, so the diag Exp runs on [96:128]
                        # into a scratch tile (96..111 junk never DMA'd out).
                        p0 = NDIAG_P0  # 112
                        nc.scalar.activation(
                            sp[:],
                            ev[:],
                            mybir.ActivationFunctionType.Ln,
                            bias=1.0,
                        )
                        nc.vector.tensor_scalar_mul(ot[0:p0], sp[0:p0], -1.0)
                        sc = dt_pool.tile([128, 1024], bf16, tag="sc")
                        nc.scalar.activation(
                            sc[96:128],
                            sp[96:128],
                            mybir.ActivationFunctionType.Exp,
                            scale=-1.0,
                        )
                        nc.gpsimd.dma_start(outT[128 : 128 + p0, cols], ot[0:p0])
                        nc.gpsimd.dma_start(
                            outT[128 + p0 : 256, cols], sc[p0:128]
                        )
    nc.compile()
    return nc


def _get_nc():
    if "nc" not in _CACHE:
        _CACHE["nc"] = _build_bass()
    return _CACHE["nc"]


def _host_prep(x, Wd, bd, Wo, bo):
    import ml_dtypes

    np_bf16 = ml_dtypes.bfloat16
    # fused rows: [Wo (240) ; -Wd (16)] — diag negated so psum holds -z and
    # sigmoid(z) = 1/(1 + e^-z) comes out of the shared exp pass
    w_dev = np.concatenate([Wo, -Wd], axis=0)  # (256, D)
    b_dev = np.concatenate([bo, -bd], axis=0)  # (256,)
    wts = np.ascontiguousarray(
        w_dev.T.reshape(KD, 128, NOUT).transpose(1, 0, 2).reshape(128, KD * NOUT)
    ).astype(np_bf16)
    biasv = np.ascontiguousarray(
        np.stack([b_dev[0:128], b_dev[128:256]], axis=1)
    ).astype(np.float32)
    in_maps = []
    for c in range(NCORES):
        xs = x[c * BC : (c + 1) * BC]  # (BC, D)
        # -> (NBLK, KD, 128, BLKC): elem (blk, kd, p, t) = xs[blk*BLKC+t, kd*128+p]
        xTc = np.ascontiguousarray(
            xs.reshape(NBLK, BLKC, KD, 128).transpose(0, 2, 3, 1)
        ).astype(np_bf16)
        in_maps.append({"xT": xTc, "wts": wts, "biasv": biasv})
    return in_maps


def _install_env_shims():
    """The agent image's `antenv` stub lacks `axon_hooks`; bass_utils imports
    it on any trace=True/BASS_TRACE run. Provide it (wired to the ctypes NTFF
    hook when available), and skip the S3 artifact upload (no egress)."""
    if "antenv.axon_hooks" in sys.modules:
        return
    import types

    try:
        import antenv
    except ImportError:
        return
    if hasattr(antenv, "axon_hooks"):
        return
    mod = types.ModuleType("antenv.axon_hooks")
    hook = [None]
    try:
        from trn_agent_boot.trn_boot import _ntff_profile_via_ctypes

        hook[0] = _ntff_profile_via_ctypes("/opt/axon/libaxon_pjrt.so")
    except Exception:
        pass
    mod.set_axon_ntff_profile_hook = lambda h: hook.__setitem__(0, h)
    mod.get_axon_ntff_profile_hook = lambda: hook[0]
    sys.modules["antenv.axon_hooks"] = mod
    antenv.axon_hooks = mod

    import concourse.bass_utils as bu

    bu.upload_artifacts = lambda tmpdir: tmpdir


def _run(in_maps, **kwargs):
    _install_env_shims()
    nc = _get_nc()
    return run_bass_kernel_spmd(nc, in_maps, list(range(NCORES)), **kwargs)


# row r of outT -> flat (i, j) position: P[i*16+j] = source row
def _out_perm():
    off_i, off_j = np.nonzero(~np.eye(K, dtype=bool))
    P = np.empty(NOUT, np.int64)
    P[off_i * K + off_j] = np.arange(K * (K - 1))
    P[np.arange(K) * (K + 1)] = K * (K - 1) + np.arange(K)
    return P


def kernel(x, Wd, bd, Wo, bo, _bench_results=None, **kwargs):
    x = np.asarray(x, np.float32)
    in_maps = _host_prep(
        x,
        np.asarray(Wd, np.float32),
        np.asarray(bd, np.float32),
        np.asarray(Wo, np.float32),
        np.asarray(bo, np.float32),
    )
    res = _run(in_maps, **kwargs)
    if _bench_results is not None:
        _bench_results.append(res)
    P = _out_perm()
    out = np.empty((B, NOUT), np.float32)
    for c in range(NCORES):
        oT = np.asarray(res.results[c]["outT"], dtype=np.float32)  # (256, BC)
        out[c * BC : (c + 1) * BC] = oT[P].T
    return out.reshape(B, K, K)
